# revision 1
# baseline (speedup 1.0000x reference)
"""Distributed TRN2 Bass kernel for nn_AgnosticResidualInteractionBlock.

Strategy (8 NeuronCores, SPMD, one Bass/Tile program on all cores):
  - Edges sharded BY RECEIVER: core k owns nodes [k*1250, (k+1)*1250).
    Receiver-partitioned local segment-sum => no collective.
  - Within a core: 10 node-blocks of 128 nodes; each block's (receiver-
    sorted) edges are packed into 17 fixed edge-tiles of 128 slots.
  - The per-edge spherical-harmonic scalars (es, ev_i, -ev_i) are folded
    into SEVEN scaled copies of the one-hot segment matrix S on the HOST;
    the PE segment matmul applies them for free. The device tensor product
    is then just five wide `w (*) x` DVE ops per tile.
  - Sender features are pre-gathered on the host FEATURE-MAJOR per tile;
    the node linear runs per tile as 4 PE matmuls (lhsT = gathered tile).
  - The radial MLP runs feature-major on PE with stationary weights.
  - Skip connection: node_attrs are PE-ones-broadcast across partitions
    and folded into the stationary operand of a chunked matmul.
  - All heavy data is bf16 (fp32 accumulation in PSUM).

kernel(**inputs) takes FULL inputs, returns (message, sc) like the
reference. Host side only re-layouts (sort/pad/gather/transpose/fold).
"""

import numpy as np

N, E, C, A, F, H = 10000, 160000, 128, 10, 8, 64
AVG_NEIGH = 16.0
NCORES = 8
NPC = N // NCORES           # 1250
NBLK = (NPC + 127) // 128   # 10 node blocks / core
TPB = 17                    # edge tiles per node block (fixed, data-checked)
NT = NBLK * TPB             # 170 edge tiles / core
ET = TPB * 128              # 2176 edge slots per block

_cache = {}
_capture = {}


def _get_jax():
    if "jax" not in _cache:
        import jax
        import jax.numpy as jnp
        _cache["jax"] = jax
        _cache["jnp"] = jnp
    return _cache["jax"], _cache["jnp"]


# ---------------------------------------------------------------- host prep
def _host_prep(inp):
    import ml_dtypes
    BF16 = ml_dtypes.bfloat16

    na = np.asarray(inp["node_attrs"], np.float32)
    nfs = np.asarray(inp["node_feats_s"], np.float32)
    nfv = np.asarray(inp["node_feats_v"], np.float32)
    ea = np.asarray(inp["edge_attrs"], np.float32)
    ef = np.asarray(inp["edge_feats"], np.float32)
    snd = np.asarray(inp["senders"]).astype(np.int64)
    rcv = np.asarray(inp["receivers"]).astype(np.int32)

    inv_sc = np.float32(1.0 / np.sqrt(C * A))
    invc = np.float32(1.0 / np.sqrt(C))
    Wlin = np.stack([
        np.asarray(inp["W_lin_s"], np.float32) * invc,
        np.asarray(inp["W_lin_v"], np.float32) * invc,
    ]).astype(BF16)                                         # [2,128,128]
    Wsc = np.stack([
        (np.asarray(inp["W_sc_s"], np.float32) * inv_sc).reshape(C, A * C),
        (np.asarray(inp["W_sc_v"], np.float32) * inv_sc).reshape(C, A * C),
    ]).astype(BF16)                                         # [2,128c,(a,o)]
    m0 = (np.asarray(inp["mlp_w0"], np.float32) / np.sqrt(np.float32(F))).astype(BF16)
    m1 = (np.asarray(inp["mlp_w1"], np.float32) / np.sqrt(np.float32(H))).astype(BF16)
    m2 = (np.asarray(inp["mlp_w2"], np.float32) / np.sqrt(np.float32(H))).astype(BF16)
    m3 = (np.asarray(inp["mlp_w3"], np.float32) / np.sqrt(np.float32(H))).astype(BF16)
    os_s = np.float32(1.0 / (np.sqrt(2 * C) * AVG_NEIGH))
    ov_s = np.float32(1.0 / (np.sqrt(3 * C) * AVG_NEIGH))
    Wo_s = np.asarray(inp["W_out_s"], np.float32) * os_s
    Wo_v = np.asarray(inp["W_out_v"], np.float32) * ov_s
    Wsa = Wo_s[:C]
    Wsb = Wo_s[C:] / np.sqrt(np.float32(3.0))
    Wva, Wvb = Wo_v[0 * C:1 * C], Wo_v[1 * C:2 * C]
    Wvc = Wo_v[2 * C:3 * C] / np.sqrt(np.float32(2.0))
    # psum/msgT chunk order: [Sa Vb0 Vb1 Vb2 Va0 Vc1 Sb Va1 Vc2 Va2 Vc0]
    Wout = np.zeros((C, 11 * C), np.float32)
    for j, Wj in enumerate([Wsa, Wvb, Wvb, Wvb, Wva, Wvc, Wsb, Wva, Wvc, Wva, Wvc]):
        Wout[:, j * C:(j + 1) * C] = Wj
    Wout = Wout.astype(BF16)

    # feature-major node features (for gather + skip connection)
    nf_t = np.zeros((4, C, N), np.float32)
    nf_t[0] = nfs.T
    for i in range(3):
        nf_t[1 + i] = nfv[:, :, i].T
    nf_tb = nf_t.astype(BF16)

    nfown_t = np.zeros((NCORES, 4, C, NBLK * 128), BF16)
    na_own = np.zeros((NCORES, 1, NBLK * A * 128), np.float32)
    for k in range(NCORES):
        nfown_t[k, :, :, :NPC] = nf_tb[:, :, k * NPC:(k + 1) * NPC]
        nak = np.zeros((NBLK * 128, A), np.float32)
        nak[:NPC] = na[k * NPC:(k + 1) * NPC]
        na_own[k, 0] = nak.reshape(NBLK, 128, A).transpose(0, 2, 1).reshape(-1)
    na_own = na_own.astype(BF16)

    # ---- edge sort & fixed tiling ----
    order = np.argsort(rcv, kind="stable")
    r_s, s_s = rcv[order], snd[order]
    ef_s, ea_s = ef[order], ea[order]
    cuts = np.searchsorted(r_s, np.arange(0, N + 1))

    # seven scaled one-hots: es, ev0, ev1, ev2, -ev0, -ev1, -ev2
    S7 = np.zeros((NCORES, NBLK, 7, 128, ET), BF16)
    ef_all = np.zeros((NCORES, NBLK, F, ET), BF16)
    xg_all = np.zeros((NCORES, NBLK, C, TPB * 4 * 128), BF16)

    for k in range(NCORES):
        for b in range(NBLK):
            n0 = k * NPC + b * 128
            n1 = min(n0 + 128, (k + 1) * NPC)
            lo, hi = cuts[n0], cuts[n1]
            nb = hi - lo
            if nb > ET:
                return None
            rl = (r_s[lo:hi] - n0).astype(np.int64)
            e_in_t = np.arange(nb) % 128
            col = (np.arange(nb) // 128) * 128 + rl
            eak = ea_s[lo:hi]
            S7[k, b, 0, e_in_t, col] = eak[:, 0]
            for i in range(3):
                S7[k, b, 1 + i, e_in_t, col] = eak[:, 1 + i]
                S7[k, b, 4 + i, e_in_t, col] = -eak[:, 1 + i]
            ef_all[k, b, :, :nb] = ef_s[lo:hi].T.astype(BF16)
            # gathered sender features, feature-major, col (t, comp, e)
            g = nf_tb[:, :, s_s[lo:hi]]                     # [4, C, nb]
            gg = np.zeros((4, C, ET), BF16)
            gg[:, :, :nb] = g
            xg_all[k, b] = gg.reshape(4, C, TPB, 128).transpose(
                1, 2, 0, 3).reshape(C, TPB * 4 * 128)
    in_maps = []
    for k in range(NCORES):
        in_maps.append({
            "nfown_t": np.asarray(nfown_t[k]),
            "na_own": np.asarray(na_own[k]),
            "S7": np.asarray(S7[k]),
            "ef_all": np.asarray(ef_all[k]),
            "xg_all": np.asarray(xg_all[k]),
            "Wlin": Wlin, "Wsc": Wsc,
            "Wm0": m0, "Wm1": m1, "Wm2": m2, "Wm3": m3,
            "Wout": Wout,
        })
    return in_maps


# ---------------------------------------------------------------- bass prog
def _build_program():
    import contextlib

    import concourse.bacc as bacc
    import concourse.tile as tile
    from concourse import mybir
    from concourse.masks import make_identity

    dt = mybir.dt
    AF = mybir.ActivationFunctionType
    OP = mybir.AluOpType

    nc = bacc.Bacc("TRN2", debug=False, enable_asserts=False)

    def ein(name, shape, dtype):
        return nc.dram_tensor(name, list(shape), dtype, kind="ExternalInput").ap()

    nfown_t = ein("nfown_t", (4, C, NBLK * 128), dt.bfloat16)
    na_own = ein("na_own", (1, NBLK * A * 128), dt.bfloat16)
    S7 = ein("S7", (NBLK, 7, 128, ET), dt.bfloat16)
    ef_all = ein("ef_all", (NBLK, F, ET), dt.bfloat16)
    xg_all = ein("xg_all", (NBLK, C, TPB * 4 * 128), dt.bfloat16)
    Wlin = ein("Wlin", (2, C, C), dt.bfloat16)
    Wsc = ein("Wsc", (2, C, A * C), dt.bfloat16)
    Wm0 = ein("Wm0", (F, H), dt.bfloat16)
    Wm1 = ein("Wm1", (H, H), dt.bfloat16)
    Wm2 = ein("Wm2", (H, H), dt.bfloat16)
    Wm3 = ein("Wm3", (H, 5 * C), dt.bfloat16)
    Wout = ein("Wout", (C, 11 * C), dt.bfloat16)

    msg_out = nc.dram_tensor("msg_out", [NBLK, 4, C, 128], dt.float32,
                             kind="ExternalOutput").ap()
    sc_out = nc.dram_tensor("sc_out", [NBLK, 4, 128, C], dt.float32,
                            kind="ExternalOutput").ap()

    with tile.TileContext(nc) as tc, contextlib.ExitStack() as ctx:
        singles = ctx.enter_context(tc.tile_pool(name="singles", bufs=1))
        tp_t = ctx.enter_context(tc.tile_pool(name="tp_t", bufs=4))
        tp_blk = ctx.enter_context(tc.tile_pool(name="tp_blk", bufs=2))
        pm = ctx.enter_context(tc.tile_pool(name="pm", bufs=1, space="PSUM"))
        pt = ctx.enter_context(tc.tile_pool(name="pt", bufs=1, space="PSUM"))
        ph = ctx.enter_context(tc.tile_pool(name="ph", bufs=1, space="PSUM"))
        px = ctx.enter_context(tc.tile_pool(name="px", bufs=1, space="PSUM"))
        ps = ctx.enter_context(tc.tile_pool(name="ps", bufs=1, space="PSUM"))

        # ------- constants in SBUF -------
        w0_sb = singles.tile([F, H], dt.bfloat16)
        nc.sync.dma_start(out=w0_sb, in_=Wm0)
        w1_sb = singles.tile([H, H], dt.bfloat16)
        nc.sync.dma_start(out=w1_sb, in_=Wm1)
        w2_sb = singles.tile([H, H], dt.bfloat16)
        nc.sync.dma_start(out=w2_sb, in_=Wm2)
        w3_sb = singles.tile([H, 5 * C], dt.bfloat16)
        nc.sync.dma_start(out=w3_sb, in_=Wm3)
        wout_sb = singles.tile([C, 11 * C], dt.bfloat16)
        nc.sync.dma_start(out=wout_sb, in_=Wout)
        wlin_sb = singles.tile([C, 2 * C], dt.bfloat16)
        nc.sync.dma_start(out=wlin_sb[:, 0:C], in_=Wlin[0])
        nc.sync.dma_start(out=wlin_sb[:, C:2 * C], in_=Wlin[1])
        wsc_sb = singles.tile([C, 2 * A * C], dt.bfloat16)
        nc.sync.dma_start(out=wsc_sb[:, :A * C], in_=Wsc[0])
        nc.sync.dma_start(out=wsc_sb[:, A * C:], in_=Wsc[1])
        na_sb = singles.tile([1, NBLK * A * 128], dt.bfloat16)
        nc.sync.dma_start(out=na_sb, in_=na_own)
        nfown_sb = singles.tile([C, 4 * NBLK * 128], dt.bfloat16)
        for c4 in range(4):
            nc.sync.dma_start(
                out=nfown_sb[:, c4 * NBLK * 128:(c4 + 1) * NBLK * 128],
                in_=nfown_t[c4])
        ident = singles.tile([128, 128], dt.bfloat16)
        make_identity(nc, ident[:])
        ones_sb = singles.tile([1, 128], dt.bfloat16)
        nc.vector.memset(ones_sb[:], 1.0)

        # ------- nab precompute: node_attrs broadcast across partitions ----
        nab_all = singles.tile([128, NBLK * A * 128], dt.bfloat16)
        for b in range(NBLK):
            for half in range(2):
                pnab = pt.tile([128, 5 * 128], dt.float32, tag="ptpw")
                for a5 in range(5):
                    o0 = (b * A + half * 5 + a5) * 128
                    nc.tensor.matmul(
                        out=pnab[:, a5 * 128:(a5 + 1) * 128],
                        lhsT=ones_sb[:], rhs=na_sb[0:1, o0:o0 + 128],
                        start=True, stop=True)
                nc.vector.tensor_copy(
                    out=nab_all[:, (b * A + half * 5) * 128:
                                (b * A + half * 5 + 5) * 128],
                    in_=pnab[:])

        # seg-matmul plan: (s7_idx, rhs_chunk_in_Mr, psum_chunk, first_writer)
        # Mr chunks: 0:P1 1:P3_0 2:P3_1 3:P3_2 4:P2 5:P4_0 6:P4_1 7:P4_2
        #            8:P5_0 9:P5_1 10:P5_2
        # psum chunks: 0:Sa 1:Vb0 2:Vb1 3:Vb2 4:Va0 5:Vc1 6:Sb 7:Va1
        #              8:Vc2 9:Va2 10:Vc0
        # NOTE: start_tensor_calc resets the ENTIRE psum bank on TRN2, so
        # exactly ONE matmul per bank carries start=True (at t==0); all other
        # chunks in that bank then accumulate onto the zeroed bank.
        # Mr chunks: 0:P2 1:P1 2:P3_0 3:P3_1 4:P3_2 5:P4_0 6:P4_1 7:P4_2
        #            8:P5_0 9:P5_1 10:P5_2
        # psum chunks: 0:Sa 1:Vb0 2:Vb1 3:Vb2 4:Va0 5:Vc1 6:Sb 7:Va1
        #              8:Vc2 9:Va2 10:Vc0
        SEG = [
            (0, 1, 0, 4, True),    # S_es @ [P1 P3*] -> [Sa Vb*] (bank0, N=512)
            (1, 0, 4, 1, True),    # S_ev0 @ P2   -> Va0  (bank1 zeroer)
            (1, 10, 5, 1, False),  # S_ev0 @ P5_2 -> Vc1 (+)
            (1, 5, 6, 1, False),   # S_ev0 @ P4_0 -> Sb
            (2, 0, 7, 1, False),   # S_ev1 @ P2   -> Va1
            (2, 8, 8, 1, True),    # S_ev1 @ P5_0 -> Vc2 (+) (bank2 zeroer)
            (2, 6, 6, 1, False),   # S_ev1 @ P4_1 -> Sb (+)
            (3, 0, 9, 1, False),   # S_ev2 @ P2   -> Va2
            (3, 9, 10, 1, False),  # S_ev2 @ P5_1 -> Vc0 (+)
            (3, 7, 6, 1, False),   # S_ev2 @ P4_2 -> Sb (+)
            (4, 9, 8, 1, False),   # S_evn0 @ P5_1 -> Vc2 (-)
            (5, 10, 10, 1, False),  # S_evn1 @ P5_2 -> Vc0 (-)
            (6, 8, 5, 1, False),   # S_evn2 @ P5_0 -> Vc1 (-)
        ]

        # ------- main loop: node blocks -------
        for b in range(NBLK):
            ef_sb = tp_blk.tile([F, ET], dt.bfloat16, tag="ef")
            nc.sync.dma_start(out=ef_sb, in_=ef_all[b])
            S_sb = tp_blk.tile([128, 7 * ET], dt.bfloat16, tag="S")
            nc.sync.dma_start(
                out=S_sb[:].rearrange("p (s e) -> p s e", s=7),
                in_=S7[b].transpose([1, 0, 2]))
            pmsg = pm.tile([128, 11 * C], dt.float32, tag="pmsg")

            # MLP runs on tile QUADS (width 512) to amortize ACT per-op cost
            for tp0 in range(0, TPB, 4):
                pw = min(4, TPB - tp0)
                W = pw * 128
                xgb_sb = tp_t.tile([C, 4 * 4 * 128], dt.bfloat16, tag="xg")
                nc.sync.dma_start(
                    out=xgb_sb[:, 0:pw * 4 * 128],
                    in_=xg_all[b][:, tp0 * 4 * 128:(tp0 + pw) * 4 * 128])
                ph0 = ph.tile([H, 512], dt.float32, tag="ph")
                nc.tensor.matmul(out=ph0[:, 0:W], lhsT=w0_sb[:],
                                 rhs=ef_sb[:, tp0 * 128:tp0 * 128 + W],
                                 start=True, stop=True)
                h0 = tp_t.tile([H, 512], dt.bfloat16, tag="h01")
                nc.scalar.activation(h0[:, 0:W], ph0[:, 0:W], AF.Silu)
                ph1 = ph.tile([H, 512], dt.float32, tag="ph")
                nc.tensor.matmul(out=ph1[:, 0:W], lhsT=w1_sb[:], rhs=h0[:, 0:W],
                                 start=True, stop=True)
                h1 = tp_t.tile([H, 512], dt.bfloat16, tag="h01")
                nc.scalar.activation(h1[:, 0:W], ph1[:, 0:W], AF.Silu)
                ph2 = ph.tile([H, 512], dt.float32, tag="ph")
                nc.tensor.matmul(out=ph2[:, 0:W], lhsT=w2_sb[:], rhs=h1[:, 0:W],
                                 start=True, stop=True)
                h2 = tp_t.tile([H, 512], dt.bfloat16, tag="h2")
                nc.scalar.activation(h2[:, 0:W], ph2[:, 0:W], AF.Silu)

                for ti in range(pw):
                    t = tp0 + ti
                    # node linear on pre-gathered sender features
                    pxl = px.tile([128, 4 * C], dt.float32, tag="pxl")
                    for c4 in range(4):
                        nc.tensor.matmul(
                            out=pxl[:, c4 * C:(c4 + 1) * C],
                            lhsT=xgb_sb[:, (ti * 4 + c4) * 128:
                                        (ti * 4 + c4 + 1) * 128],
                            rhs=wlin_sb[:, 0:C] if c4 == 0 else wlin_sb[:, C:2 * C],
                            start=True, stop=True)
                    xt = tp_t.tile([128, 4 * C], dt.bfloat16, tag="xt")
                    nc.scalar.copy(out=xt[:], in_=pxl[:])

                    ptpw = pt.tile([128, 5 * C], dt.float32, tag="ptpw")
                    h2t = h2[:, ti * 128:(ti + 1) * 128]
                    nc.tensor.matmul(out=ptpw[:, 0:512], lhsT=h2t,
                                     rhs=w3_sb[:, 0:512], start=True, stop=True)
                    nc.tensor.matmul(out=ptpw[:, 512:640], lhsT=h2t,
                                     rhs=w3_sb[:, 512:640], start=True, stop=True)
                    tpwE = tp_t.tile([128, 5 * C], dt.bfloat16, tag="tpwE")
                    nc.vector.tensor_copy(out=tpwE[:, 0:2 * C], in_=ptpw[:, 0:2 * C])
                    nc.scalar.copy(out=tpwE[:, 2 * C:5 * C], in_=ptpw[:, 2 * C:5 * C])

                    # Mr = [P2 | P1 | P3* | P4* | P5*] ; P_i = w_i (*) x
                    Mr = tp_t.tile([128, 11 * C], dt.bfloat16, tag="Mr")
                    xs = xt[:, 0:C]
                    xv = xt[:, C:4 * C]
                    nc.vector.tensor_tensor(out=Mr[:, 0:C], in0=tpwE[:, C:2 * C],
                                            in1=xs, op=OP.mult)
                    nc.vector.tensor_tensor(out=Mr[:, C:2 * C], in0=tpwE[:, 0:C],
                                            in1=xs, op=OP.mult)
                    nc.vector.tensor_tensor(
                        out=Mr[:, 2 * C:11 * C].rearrange(
                            "p (w r c) -> p w r c", w=3, r=3),
                        in0=tpwE[:, 2 * C:5 * C].rearrange(
                            "p (w c) -> p w c", w=3).unsqueeze(2).to_broadcast(
                            [128, 3, 3, C]),
                        in1=xv.rearrange("p (r c) -> p r c", r=3).unsqueeze(
                            1).to_broadcast([128, 3, 3, C]),
                        op=OP.mult)

                    # segment matmuls with scaled one-hots
                    for (s7i, rc, pc, nch, first) in SEG:
                        nc.tensor.matmul(
                            out=pmsg[:, pc * C:(pc + nch) * C],
                            lhsT=S_sb[:, (s7i * TPB + t) * 128:
                                      (s7i * TPB + t + 1) * 128],
                            rhs=Mr[:, rc * C:(rc + nch) * C],
                            start=(t == 0 and first), stop=(t == TPB - 1),
                            skip_group_check=True)

            # ---- block tail: evict msg, transpose, output linear ----
            msg_sb = tp_blk.tile([128, 11 * C], dt.bfloat16, tag="msg")
            nc.vector.tensor_copy(out=msg_sb[:, 0:6 * C], in_=pmsg[:, 0:6 * C])
            nc.scalar.copy(out=msg_sb[:, 6 * C:11 * C], in_=pmsg[:, 6 * C:11 * C])
            msgT = tp_blk.tile([128, 11 * C], dt.bfloat16, tag="msgT")
            for j in range(11):
                ptr = ps.tile([128, 512], dt.bfloat16, tag="sm")
                nc.tensor.transpose(out=ptr[:, 0:C],
                                    in_=msg_sb[:, j * C:(j + 1) * C],
                                    identity=ident[:])
                if j % 2 == 0:
                    nc.vector.tensor_copy(out=msgT[:, j * C:(j + 1) * C],
                                          in_=ptr[:, 0:C])
                else:
                    nc.scalar.copy(out=msgT[:, j * C:(j + 1) * C], in_=ptr[:, 0:C])
            outmsg = tp_blk.tile([128, 4 * C], dt.float32, tag="outmsg")
            CH = {0: (0, 6), 1: (4, 1, 10), 2: (7, 2, 5), 3: (9, 3, 8)}
            for c4 in range(4):
                chunks = CH[c4]
                pout = ps.tile([128, 512], dt.float32, tag="sm")
                for ji, j in enumerate(chunks):
                    nc.tensor.matmul(
                        out=pout[:, 0:C],
                        lhsT=wout_sb[:, j * C:(j + 1) * C],
                        rhs=msgT[:, j * C:(j + 1) * C],
                        start=(ji == 0), stop=(ji == len(chunks) - 1))
                if c4 % 2 == 0:
                    nc.vector.tensor_copy(out=outmsg[:, c4 * C:(c4 + 1) * C],
                                          in_=pout[:, 0:C])
                else:
                    nc.scalar.copy(out=outmsg[:, c4 * C:(c4 + 1) * C],
                                   in_=pout[:, 0:C])
            nc.sync.dma_start(out=msg_out[b].transpose([1, 0, 2]),
                              in_=outmsg[:].rearrange("p (c n) -> p c n", c=4))

            # ---- skip connection for this block ----
            outsc = tp_blk.tile([128, 4 * C], dt.float32, tag="outsc")
            for c4 in range(4):
                X = tp_blk.tile([C, A * 128], dt.bfloat16, tag="X")
                nfb = nfown_sb[:, (c4 * NBLK + b) * 128:(c4 * NBLK + b + 1) * 128]
                nc.gpsimd.tensor_tensor(
                    out=X[:].rearrange("p (a n) -> p a n", a=A),
                    in0=nfb.unsqueeze(1).to_broadcast([C, A, 128]),
                    in1=nab_all[:, b * A * 128:(b + 1) * A * 128].rearrange(
                        "p (a n) -> p a n", a=A),
                    op=OP.mult)
                psc = ps.tile([128, 512], dt.float32, tag="sm")
                wsc_c = wsc_sb[:, 0:A * C] if c4 == 0 else wsc_sb[:, A * C:]
                for a in range(A):
                    nc.tensor.matmul(
                        out=psc[:, 0:C],
                        lhsT=X[:, a * 128:(a + 1) * 128],
                        rhs=wsc_c[:, a * C:(a + 1) * C],
                        start=(a == 0), stop=(a == A - 1))
                if c4 % 2 == 0:
                    nc.scalar.copy(out=outsc[:, c4 * C:(c4 + 1) * C],
                                   in_=psc[:, 0:C])
                else:
                    nc.vector.tensor_copy(out=outsc[:, c4 * C:(c4 + 1) * C],
                                          in_=psc[:, 0:C])
            nc.sync.dma_start(out=sc_out[b].transpose([1, 0, 2]),
                              in_=outsc[:].rearrange("p (c o) -> p c o", c=4))

    nc.compile()
    nc.finalize()
    return nc


# ------------------------------------------------------------- pjrt runner
def _prepare_fn(nc):
    """Build a reusable jitted shard_map callable over the bass program."""
    jax, _ = _get_jax()
    from jax.sharding import Mesh, PartitionSpec
    try:
        from jax.experimental.shard_map import shard_map
    except ImportError:
        from jax.shard_map import shard_map
    from concourse import bass2jax, mybir
    bass2jax.install_neuronx_cc_hook()

    partition_name = (nc.partition_id_tensor.name
                      if nc.partition_id_tensor else None)
    in_names, out_names, out_avals, zero_shapes = [], [], [], []
    for alloc in nc.m.functions[0].allocations:
        if not isinstance(alloc, mybir.MemoryLocationSet):
            continue
        name = alloc.memorylocations[0].name
        if alloc.kind == "ExternalInput":
            if name != partition_name:
                in_names.append(name)
        elif alloc.kind == "ExternalOutput":
            shape = tuple(alloc.tensor_shape)
            dtype = mybir.dt.np(alloc.dtype)
            out_names.append(name)
            out_avals.append(jax.core.ShapedArray(shape, dtype))
            zero_shapes.append((shape, dtype))
    all_in = list(in_names) + list(out_names)
    if partition_name is not None:
        all_in.append(partition_name)

    def _body(*args):
        operands = list(args)
        if partition_name is not None:
            operands.append(bass2jax.partition_id_tensor())
        outs = bass2jax._bass_exec_p.bind(
            *operands,
            out_avals=tuple(out_avals),
            in_names=tuple(all_in),
            out_names=tuple(out_names),
            lowering_input_output_aliases=(),
            sim_require_finite=False,
            sim_require_nnan=False,
            nc=nc,
        )
        return tuple(outs)

    devices = jax.devices()[:NCORES]
    mesh = Mesh(np.asarray(devices), ("core",))
    nin = len(in_names) + len(zero_shapes)
    fn = jax.jit(shard_map(
        _body, mesh=mesh,
        in_specs=(PartitionSpec("core"),) * nin,
        out_specs=(PartitionSpec("core"),) * len(out_names),
        check_rep=False))
    return fn, in_names, out_names, zero_shapes


def _concat_args(in_maps, in_names, zero_shapes):
    concat_in = [
        np.concatenate([np.asarray(in_maps[c][nm]) for c in range(NCORES)], 0)
        for nm in in_names
    ]
    concat_zero = [np.zeros((NCORES * s[0], *s[1:]), d) for s, d in zero_shapes]
    return tuple(concat_in + concat_zero)


# ------------------------------------------------------------------ driver
def _assemble(out_map):
    msg_out = np.asarray(out_map["msg_out"]).reshape(NCORES, NBLK, 4, C, 128)
    sc_out = np.asarray(out_map["sc_out"]).reshape(NCORES, NBLK, 4, 128, C)
    message = np.zeros((N, 4 * C), np.float32)
    sc = np.zeros((N, 4 * C), np.float32)
    for k in range(NCORES):
        for b in range(NBLK):
            n0 = k * NPC + b * 128
            n1 = min(n0 + 128, (k + 1) * NPC)
            nn = n1 - n0
            message[n0:n1, 0:C] = msg_out[k, b, 0, :, :nn].T
            for i in range(3):
                message[n0:n1, C + i::3] = msg_out[k, b, 1 + i, :, :nn].T
            sc[n0:n1, 0:C] = sc_out[k, b, 0, :nn, :]
            for i in range(3):
                sc[n0:n1, C + i::3] = sc_out[k, b, 1 + i, :nn, :]
    return message, sc


def _numpy_fallback(inp):
    na = np.asarray(inp["node_attrs"], np.float32)
    nfs = np.asarray(inp["node_feats_s"], np.float32)
    nfv = np.asarray(inp["node_feats_v"], np.float32)
    ea = np.asarray(inp["edge_attrs"], np.float32)
    ef = np.asarray(inp["edge_feats"], np.float32)
    snd = np.asarray(inp["senders"]).astype(np.int64)
    rcv = np.asarray(inp["receivers"]).astype(np.int64)
    inv = np.float32(1.0 / np.sqrt(C * A))
    invc = np.float32(1.0 / np.sqrt(C))
    tp_s = (nfs[:, :, None] * na[:, None, :]).reshape(N, C * A)
    sc_s = tp_s @ np.asarray(inp["W_sc_s"], np.float32) * inv
    tp_v = (nfv[:, :, None, :] * na[:, None, :, None]).reshape(N, C * A, 3)
    sc_v = np.einsum("nki,ko->noi", tp_v,
                     np.asarray(inp["W_sc_v"], np.float32)) * inv
    x_s = nfs @ np.asarray(inp["W_lin_s"], np.float32) * invc
    x_v = np.einsum("nci,co->noi", nfv, np.asarray(inp["W_lin_v"], np.float32)) * invc

    def silu(x):
        return x / (1.0 + np.exp(-x))
    h = silu(ef @ np.asarray(inp["mlp_w0"], np.float32) / np.sqrt(np.float32(F)))
    h = silu(h @ np.asarray(inp["mlp_w1"], np.float32) / np.sqrt(np.float32(H)))
    h = silu(h @ np.asarray(inp["mlp_w2"], np.float32) / np.sqrt(np.float32(H)))
    tpw = h @ np.asarray(inp["mlp_w3"], np.float32) / np.sqrt(np.float32(H))
    w1, w2, w3, w4, w5 = np.split(tpw, 5, axis=1)
    xs, xv = x_s[snd], x_v[snd]
    es, ev = ea[:, 0:1], ea[:, 1:4]
    m0a = w1 * xs * es
    m1a = (w2 * xs)[:, :, None] * ev[:, None, :]
    m1b = w3[:, :, None] * xv * es[:, :, None]
    m0b = w4 * np.einsum("eci,ei->ec", xv, ev) / np.sqrt(np.float32(3))
    m1c = w5[:, :, None] * np.cross(xv, ev[:, None, :]) / np.sqrt(np.float32(2))
    mid_s = np.concatenate([m0a, m0b], axis=1)
    mid_v = np.concatenate([m1a, m1b, m1c], axis=1)
    msg_s = np.zeros((N, 2 * C), np.float32)
    np.add.at(msg_s, rcv, mid_s)
    msg_v = np.zeros((N, 3 * C, 3), np.float32)
    np.add.at(msg_v, rcv, mid_v)
    out_s = (msg_s @ np.asarray(inp["W_out_s"], np.float32)
             / np.sqrt(np.float32(2 * C)) / AVG_NEIGH)
    out_v = (np.einsum("nki,ko->noi", msg_v, np.asarray(inp["W_out_v"], np.float32))
             / np.sqrt(np.float32(3 * C)) / AVG_NEIGH)
    message = np.concatenate([out_s, out_v.reshape(N, C * 3)], axis=1)
    sc = np.concatenate([sc_s, sc_v.reshape(N, C * 3)], axis=1)
    return message.astype(np.float32), sc.astype(np.float32)


def kernel(node_attrs, node_feats_s, node_feats_v, edge_attrs, edge_feats,
           W_sc_s, W_sc_v, W_lin_s, W_lin_v,
           mlp_w0, mlp_w1, mlp_w2, mlp_w3,
           W_out_s, W_out_v, senders, receivers):
    inp = dict(node_attrs=node_attrs, node_feats_s=node_feats_s,
               node_feats_v=node_feats_v, edge_attrs=edge_attrs,
               edge_feats=edge_feats, W_sc_s=W_sc_s, W_sc_v=W_sc_v,
               W_lin_s=W_lin_s, W_lin_v=W_lin_v, mlp_w0=mlp_w0, mlp_w1=mlp_w1,
               mlp_w2=mlp_w2, mlp_w3=mlp_w3, W_out_s=W_out_s, W_out_v=W_out_v,
               senders=senders, receivers=receivers)
    try:
        in_maps = _host_prep(inp)
        if in_maps is None:
            raise RuntimeError("edge tile overflow; falling back")
        if "nc" not in _cache:
            _cache["nc"] = _build_program()
        if "fn" not in _cache:
            fn, in_names, out_names, zero_shapes = _prepare_fn(_cache["nc"])
            _cache.update(fn=fn, in_names=in_names, out_names=out_names,
                          zero_shapes=zero_shapes)
        args = _concat_args(in_maps, _cache["in_names"], _cache["zero_shapes"])
        out = _cache["fn"](*args)
        out = [np.asarray(o) for o in out]
        _capture["fn"] = _cache["fn"]
        _capture["args"] = args
        out_map = {nm: out[i] for i, nm in enumerate(_cache["out_names"])}
        return _assemble(out_map)
    except Exception:
        import traceback
        traceback.print_exc()
        return _numpy_fallback(inp)


if __name__ == "__main__":
    import jax as _j
    with _j.default_device(_j.devices("cpu")[0]):
        import reference
        inputs = {k: np.asarray(v) for k, v in reference.setup_inputs().items()}
        exp_msg, exp_sc = (np.asarray(x) for x in reference.reference(**inputs))
    act_msg, act_sc = kernel(**inputs)
    for name, e, a in (("message", exp_msg, act_msg), ("sc", exp_sc, act_sc)):
        err = np.abs(a - e).max() / (np.abs(e).max() + 1e-9)
        print(f"{name}: rel_err={err:.3e}", flush=True)



# revision 4
# speedup vs baseline: 1.2569x; 1.2569x over previous
"""Distributed TRN2 Bass kernel for nn_AgnosticResidualInteractionBlock.

Strategy (8 NeuronCores, SPMD, one Bass/Tile program on all cores):
  - Edges sharded BY RECEIVER: core k owns nodes [k*1250, (k+1)*1250).
    Receiver-partitioned local segment-sum => no collective.
  - Within a core: 10 node-blocks of 128 nodes; each block's (receiver-
    sorted) edges are packed into 17 fixed edge-tiles of 128 slots.
  - The per-edge spherical-harmonic scalars (es, ev_i, -ev_i) are folded
    into SEVEN scaled copies of the one-hot segment matrix S on the HOST;
    the PE segment matmul applies them for free. The device tensor product
    is then just five wide `w (*) x` DVE ops per tile.
  - Sender features are pre-gathered on the host FEATURE-MAJOR per tile;
    the node linear runs per tile as 4 PE matmuls (lhsT = gathered tile).
  - The radial MLP runs feature-major on PE with stationary weights.
  - Skip connection: node_attrs are PE-ones-broadcast across partitions
    and folded into the stationary operand of a chunked matmul.
  - All heavy data is bf16 (fp32 accumulation in PSUM).

kernel(**inputs) takes FULL inputs, returns (message, sc) like the
reference. Host side only re-layouts (sort/pad/gather/transpose/fold).
"""

import numpy as np

N, E, C, A, F, H = 10000, 160000, 128, 10, 8, 64
AVG_NEIGH = 16.0
NCORES = 8
NPC = N // NCORES           # 1250
NBLK = (NPC + 127) // 128   # 10 node blocks / core
TPB = 17                    # edge tiles per node block (fixed, data-checked)
NT = NBLK * TPB             # 170 edge tiles / core
ET = TPB * 128              # 2176 edge slots per block

_cache = {}
_capture = {}


def _get_jax():
    if "jax" not in _cache:
        import jax
        import jax.numpy as jnp
        _cache["jax"] = jax
        _cache["jnp"] = jnp
    return _cache["jax"], _cache["jnp"]


# ---------------------------------------------------------------- host prep
def _host_prep(inp):
    import ml_dtypes
    BF16 = ml_dtypes.bfloat16

    na = np.asarray(inp["node_attrs"], np.float32)
    nfs = np.asarray(inp["node_feats_s"], np.float32)
    nfv = np.asarray(inp["node_feats_v"], np.float32)
    ea = np.asarray(inp["edge_attrs"], np.float32)
    ef = np.asarray(inp["edge_feats"], np.float32)
    snd = np.asarray(inp["senders"]).astype(np.int64)
    rcv = np.asarray(inp["receivers"]).astype(np.int32)

    inv_sc = np.float32(1.0 / np.sqrt(C * A))
    invc = np.float32(1.0 / np.sqrt(C))
    Wlin = np.stack([
        np.asarray(inp["W_lin_s"], np.float32) * invc,
        np.asarray(inp["W_lin_v"], np.float32) * invc,
    ]).astype(BF16)                                         # [2,128,128]
    Wsc = np.stack([
        (np.asarray(inp["W_sc_s"], np.float32) * inv_sc).reshape(C, A * C),
        (np.asarray(inp["W_sc_v"], np.float32) * inv_sc).reshape(C, A * C),
    ]).astype(BF16)                                         # [2,128c,(a,o)]
    m0 = (np.asarray(inp["mlp_w0"], np.float32) / np.sqrt(np.float32(F))).astype(BF16)
    m1 = (np.asarray(inp["mlp_w1"], np.float32) / np.sqrt(np.float32(H))).astype(BF16)
    m2 = (np.asarray(inp["mlp_w2"], np.float32) / np.sqrt(np.float32(H))).astype(BF16)
    m3 = (np.asarray(inp["mlp_w3"], np.float32) / np.sqrt(np.float32(H))).astype(BF16)
    os_s = np.float32(1.0 / (np.sqrt(2 * C) * AVG_NEIGH))
    ov_s = np.float32(1.0 / (np.sqrt(3 * C) * AVG_NEIGH))
    Wo_s = np.asarray(inp["W_out_s"], np.float32) * os_s
    Wo_v = np.asarray(inp["W_out_v"], np.float32) * ov_s
    Wsa = Wo_s[:C]
    Wsb = Wo_s[C:] / np.sqrt(np.float32(3.0))
    Wva, Wvb = Wo_v[0 * C:1 * C], Wo_v[1 * C:2 * C]
    Wvc = Wo_v[2 * C:3 * C] / np.sqrt(np.float32(2.0))
    # psum/msgT chunk order: [Sa Vb0 Vb1 Vb2 Va0 Vc1 Sb Va1 Vc2 Va2 Vc0]
    Wout = np.zeros((C, 11 * C), np.float32)
    for j, Wj in enumerate([Wsa, Wvb, Wvb, Wvb, Wva, Wvc, Wsb, Wva, Wvc, Wva, Wvc]):
        Wout[:, j * C:(j + 1) * C] = Wj
    Wout = Wout.astype(BF16)

    # feature-major node features (for gather + skip connection)
    nf_t = np.zeros((4, C, N), np.float32)
    nf_t[0] = nfs.T
    for i in range(3):
        nf_t[1 + i] = nfv[:, :, i].T
    nf_tb = nf_t.astype(BF16)

    nfown_t = np.zeros((NCORES, 4, C, NBLK * 128), BF16)
    na_own = np.zeros((NCORES, 1, NBLK * A * 128), np.float32)
    for k in range(NCORES):
        nfown_t[k, :, :, :NPC] = nf_tb[:, :, k * NPC:(k + 1) * NPC]
        nak = np.zeros((NBLK * 128, A), np.float32)
        nak[:NPC] = na[k * NPC:(k + 1) * NPC]
        na_own[k, 0] = nak.reshape(NBLK, 128, A).transpose(0, 2, 1).reshape(-1)
    na_own = na_own.astype(BF16)

    # ---- edge sort & fixed tiling ----
    order = np.argsort(rcv, kind="stable")
    r_s, s_s = rcv[order], snd[order]
    ef_s, ea_s = ef[order], ea[order]
    cuts = np.searchsorted(r_s, np.arange(0, N + 1))

    # seven scaled one-hots: es, ev0, ev1, ev2, -ev0, -ev1, -ev2
    S7 = np.zeros((NCORES, NBLK, 7, 128, ET), BF16)
    ef_all = np.zeros((NCORES, NBLK, F, ET), BF16)
    xg_all = np.zeros((NCORES, NBLK, C, TPB * 4 * 128), BF16)

    for k in range(NCORES):
        for b in range(NBLK):
            n0 = k * NPC + b * 128
            n1 = min(n0 + 128, (k + 1) * NPC)
            lo, hi = cuts[n0], cuts[n1]
            nb = hi - lo
            if nb > ET:
                return None
            rl = (r_s[lo:hi] - n0).astype(np.int64)
            e_in_t = np.arange(nb) % 128
            col = (np.arange(nb) // 128) * 128 + rl
            eak = ea_s[lo:hi]
            S7[k, b, 0, e_in_t, col] = eak[:, 0]
            for i in range(3):
                S7[k, b, 1 + i, e_in_t, col] = eak[:, 1 + i]
                S7[k, b, 4 + i, e_in_t, col] = -eak[:, 1 + i]
            ef_all[k, b, :, :nb] = ef_s[lo:hi].T.astype(BF16)
            # gathered sender features, feature-major, col (t, comp, e)
            g = nf_tb[:, :, s_s[lo:hi]]                     # [4, C, nb]
            gg = np.zeros((4, C, ET), BF16)
            gg[:, :, :nb] = g
            xg_all[k, b] = gg.reshape(4, C, TPB, 128).transpose(
                1, 2, 0, 3).reshape(C, TPB * 4 * 128)
    # single packed input blob per core: fewer runtime args = less
    # per-launch marshaling overhead
    wtail = np.concatenate([
        Wlin.ravel(), Wsc.ravel(), m0.ravel(), m1.ravel(), m2.ravel(),
        m3.ravel(), Wout.ravel()])
    in_maps = []
    for k in range(NCORES):
        blob = np.concatenate([
            S7[k].ravel(), xg_all[k].ravel(), ef_all[k].ravel(),
            nfown_t[k].ravel(), na_own[k].ravel(), wtail])
        in_maps.append({"blob": np.ascontiguousarray(blob)})
    return in_maps


# ---------------------------------------------------------------- bass prog
def _build_program():
    import contextlib

    import concourse.bacc as bacc
    import concourse.tile as tile
    from concourse import mybir
    from concourse.masks import make_identity

    dt = mybir.dt
    AF = mybir.ActivationFunctionType
    OP = mybir.AluOpType

    nc = bacc.Bacc("TRN2", debug=False, enable_asserts=False)

    sizes = [NBLK * 7 * 128 * ET, NBLK * C * TPB * 4 * 128, NBLK * F * ET,
             4 * C * NBLK * 128, NBLK * A * 128, 2 * C * C, 2 * C * A * C,
             F * H, H * H, H * H, H * 5 * C, C * 11 * C]
    total_in = sum(sizes)
    blob = nc.dram_tensor("blob", [total_in], dt.bfloat16,
                          kind="ExternalInput").ap()
    _off = [0]

    def take(*shape):
        n = int(np.prod(shape))
        v = blob[_off[0]:_off[0] + n]
        if len(shape) > 1:
            letters = "abcdefg"[:len(shape)]
            pat = f"({' '.join(letters)}) -> {' '.join(letters)}"
            kw = {letters[i]: int(shape[i]) for i in range(len(shape) - 1)}
            v = v.rearrange(pat, **kw)
        _off[0] += n
        return v

    S7 = take(NBLK, 7, 128, ET)
    xg_all = take(NBLK, C, TPB * 4 * 128)
    ef_all = take(NBLK, F, ET)
    nfown_t = take(4, C, NBLK * 128)
    na_own = take(1, NBLK * A * 128)
    Wlin = take(2, C, C)
    Wsc = take(2, C, A * C)
    Wm0 = take(F, H)
    Wm1 = take(H, H)
    Wm2 = take(H, H)
    Wm3 = take(H, 5 * C)
    Wout = take(C, 11 * C)
    assert _off[0] == total_in

    out_blob = nc.dram_tensor("out_blob", [2 * NBLK * 4 * C * 128], dt.float32,
                              kind="ExternalOutput").ap()
    half = NBLK * 4 * C * 128
    msg_out = out_blob[0:half].rearrange("(b c p n) -> b c p n",
                                         b=NBLK, c=4, p=C)
    sc_out = out_blob[half:2 * half].rearrange("(b c p n) -> b c p n",
                                               b=NBLK, c=4, p=128)

    with tile.TileContext(nc) as tc, contextlib.ExitStack() as ctx:
        singles = ctx.enter_context(tc.tile_pool(name="singles", bufs=1))
        tp_t = ctx.enter_context(tc.tile_pool(name="tp_t", bufs=4))
        tp_blk = ctx.enter_context(tc.tile_pool(name="tp_blk", bufs=2))
        pm = ctx.enter_context(tc.tile_pool(name="pm", bufs=1, space="PSUM"))
        pt = ctx.enter_context(tc.tile_pool(name="pt", bufs=1, space="PSUM"))
        ph = ctx.enter_context(tc.tile_pool(name="ph", bufs=1, space="PSUM"))
        px = ctx.enter_context(tc.tile_pool(name="px", bufs=1, space="PSUM"))
        ps = ctx.enter_context(tc.tile_pool(name="ps", bufs=1, space="PSUM"))

        # ------- constants in SBUF -------
        w0_sb = singles.tile([F, H], dt.bfloat16)
        nc.sync.dma_start(out=w0_sb, in_=Wm0)
        w1_sb = singles.tile([H, H], dt.bfloat16)
        nc.sync.dma_start(out=w1_sb, in_=Wm1)
        w2_sb = singles.tile([H, H], dt.bfloat16)
        nc.sync.dma_start(out=w2_sb, in_=Wm2)
        w3_sb = singles.tile([H, 5 * C], dt.bfloat16)
        nc.sync.dma_start(out=w3_sb, in_=Wm3)
        wout_sb = singles.tile([C, 11 * C], dt.bfloat16)
        nc.sync.dma_start(out=wout_sb, in_=Wout)
        wlin_sb = singles.tile([C, 2 * C], dt.bfloat16)
        nc.sync.dma_start(out=wlin_sb[:, 0:C], in_=Wlin[0])
        nc.sync.dma_start(out=wlin_sb[:, C:2 * C], in_=Wlin[1])
        wsc_sb = singles.tile([C, 2 * A * C], dt.bfloat16)
        nc.sync.dma_start(out=wsc_sb[:, :A * C], in_=Wsc[0])
        nc.sync.dma_start(out=wsc_sb[:, A * C:], in_=Wsc[1])
        na_sb = singles.tile([1, NBLK * A * 128], dt.bfloat16)
        nc.sync.dma_start(out=na_sb, in_=na_own)
        nfown_sb = singles.tile([C, 4 * NBLK * 128], dt.bfloat16)
        for c4 in range(4):
            nc.sync.dma_start(
                out=nfown_sb[:, c4 * NBLK * 128:(c4 + 1) * NBLK * 128],
                in_=nfown_t[c4])
        ident = singles.tile([128, 128], dt.bfloat16)
        make_identity(nc, ident[:])
        ones_sb = singles.tile([1, 128], dt.bfloat16)
        nc.vector.memset(ones_sb[:], 1.0)

        # ------- nab precompute: node_attrs broadcast across partitions ----
        nab_all = singles.tile([128, NBLK * A * 128], dt.bfloat16)
        for b in range(NBLK):
            for half in range(2):
                pnab = pt.tile([128, 5 * 128], dt.float32, tag="ptpw")
                for a5 in range(5):
                    o0 = (b * A + half * 5 + a5) * 128
                    nc.tensor.matmul(
                        out=pnab[:, a5 * 128:(a5 + 1) * 128],
                        lhsT=ones_sb[:], rhs=na_sb[0:1, o0:o0 + 128],
                        start=True, stop=True)
                nc.vector.tensor_copy(
                    out=nab_all[:, (b * A + half * 5) * 128:
                                (b * A + half * 5 + 5) * 128],
                    in_=pnab[:])

        # seg-matmul plan: (s7_idx, rhs_chunk_in_Mr, psum_chunk, first_writer)
        # Mr chunks: 0:P1 1:P3_0 2:P3_1 3:P3_2 4:P2 5:P4_0 6:P4_1 7:P4_2
        #            8:P5_0 9:P5_1 10:P5_2
        # psum chunks: 0:Sa 1:Vb0 2:Vb1 3:Vb2 4:Va0 5:Vc1 6:Sb 7:Va1
        #              8:Vc2 9:Va2 10:Vc0
        # NOTE: start_tensor_calc resets the ENTIRE psum bank on TRN2, so
        # exactly ONE matmul per bank carries start=True (at t==0); all other
        # chunks in that bank then accumulate onto the zeroed bank.
        # Mr chunks: 0:P2 1:P1 2:P3_0 3:P3_1 4:P3_2 5:P4_0 6:P4_1 7:P4_2
        #            8:P5_0 9:P5_1 10:P5_2
        # psum chunks: 0:Sa 1:Vb0 2:Vb1 3:Vb2 4:Va0 5:Vc1 6:Sb 7:Va1
        #              8:Vc2 9:Va2 10:Vc0
        SEG = [
            (0, 1, 0, 4, True),    # S_es @ [P1 P3*] -> [Sa Vb*] (bank0, N=512)
            (1, 0, 4, 1, True),    # S_ev0 @ P2   -> Va0  (bank1 zeroer)
            (1, 10, 5, 1, False),  # S_ev0 @ P5_2 -> Vc1 (+)
            (1, 5, 6, 1, False),   # S_ev0 @ P4_0 -> Sb
            (2, 0, 7, 1, False),   # S_ev1 @ P2   -> Va1
            (2, 8, 8, 1, True),    # S_ev1 @ P5_0 -> Vc2 (+) (bank2 zeroer)
            (2, 6, 6, 1, False),   # S_ev1 @ P4_1 -> Sb (+)
            (3, 0, 9, 1, False),   # S_ev2 @ P2   -> Va2
            (3, 9, 10, 1, False),  # S_ev2 @ P5_1 -> Vc0 (+)
            (3, 7, 6, 1, False),   # S_ev2 @ P4_2 -> Sb (+)
            (4, 9, 8, 1, False),   # S_evn0 @ P5_1 -> Vc2 (-)
            (5, 10, 10, 1, False),  # S_evn1 @ P5_2 -> Vc0 (-)
            (6, 8, 5, 1, False),   # S_evn2 @ P5_0 -> Vc1 (-)
        ]

        # ------- main loop: node blocks -------
        for b in range(NBLK):
            ef_sb = tp_blk.tile([F, ET], dt.bfloat16, tag="ef")
            nc.sync.dma_start(out=ef_sb, in_=ef_all[b])
            S_sb = tp_blk.tile([128, 7 * ET], dt.bfloat16, tag="S")
            nc.sync.dma_start(
                out=S_sb[:].rearrange("p (s e) -> p s e", s=7),
                in_=S7[b].transpose([1, 0, 2]))
            pmsg = pm.tile([128, 11 * C], dt.float32, tag="pmsg")

            # MLP runs on tile QUADS (width 512) to amortize ACT per-op cost
            for tp0 in range(0, TPB, 4):
                pw = min(4, TPB - tp0)
                W = pw * 128
                xgb_sb = tp_t.tile([C, 4 * 4 * 128], dt.bfloat16, tag="xg")
                nc.sync.dma_start(
                    out=xgb_sb[:, 0:pw * 4 * 128],
                    in_=xg_all[b][:, tp0 * 4 * 128:(tp0 + pw) * 4 * 128])
                ph0 = ph.tile([H, 512], dt.float32, tag="ph")
                nc.tensor.matmul(out=ph0[:, 0:W], lhsT=w0_sb[:],
                                 rhs=ef_sb[:, tp0 * 128:tp0 * 128 + W],
                                 start=True, stop=True)
                h0 = tp_t.tile([H, 512], dt.bfloat16, tag="h01")
                nc.scalar.activation(h0[:, 0:W], ph0[:, 0:W], AF.Silu)
                ph1 = ph.tile([H, 512], dt.float32, tag="ph")
                nc.tensor.matmul(out=ph1[:, 0:W], lhsT=w1_sb[:], rhs=h0[:, 0:W],
                                 start=True, stop=True)
                h1 = tp_t.tile([H, 512], dt.bfloat16, tag="h01")
                nc.scalar.activation(h1[:, 0:W], ph1[:, 0:W], AF.Silu)
                ph2 = ph.tile([H, 512], dt.float32, tag="ph")
                nc.tensor.matmul(out=ph2[:, 0:W], lhsT=w2_sb[:], rhs=h1[:, 0:W],
                                 start=True, stop=True)
                h2 = tp_t.tile([H, 512], dt.bfloat16, tag="h2")
                nc.scalar.activation(h2[:, 0:W], ph2[:, 0:W], AF.Silu)

                for ti in range(pw):
                    t = tp0 + ti
                    # node linear on pre-gathered sender features
                    pxl = px.tile([128, 4 * C], dt.float32, tag="pxl")
                    for c4 in range(4):
                        nc.tensor.matmul(
                            out=pxl[:, c4 * C:(c4 + 1) * C],
                            lhsT=xgb_sb[:, (ti * 4 + c4) * 128:
                                        (ti * 4 + c4 + 1) * 128],
                            rhs=wlin_sb[:, 0:C] if c4 == 0 else wlin_sb[:, C:2 * C],
                            start=True, stop=True)
                    xt = tp_t.tile([128, 4 * C], dt.bfloat16, tag="xt")
                    nc.scalar.copy(out=xt[:], in_=pxl[:])

                    ptpw = pt.tile([128, 5 * C], dt.float32, tag="ptpw")
                    h2t = h2[:, ti * 128:(ti + 1) * 128]
                    nc.tensor.matmul(out=ptpw[:, 0:512], lhsT=h2t,
                                     rhs=w3_sb[:, 0:512], start=True, stop=True)
                    nc.tensor.matmul(out=ptpw[:, 512:640], lhsT=h2t,
                                     rhs=w3_sb[:, 512:640], start=True, stop=True)
                    tpwE = tp_t.tile([128, 5 * C], dt.bfloat16, tag="tpwE")
                    nc.vector.tensor_copy(out=tpwE[:, 0:2 * C], in_=ptpw[:, 0:2 * C])
                    nc.scalar.copy(out=tpwE[:, 2 * C:5 * C], in_=ptpw[:, 2 * C:5 * C])

                    # Mr = [P2 | P1 | P3* | P4* | P5*] ; P_i = w_i (*) x
                    Mr = tp_t.tile([128, 11 * C], dt.bfloat16, tag="Mr")
                    xs = xt[:, 0:C]
                    xv = xt[:, C:4 * C]
                    nc.vector.tensor_tensor(out=Mr[:, 0:C], in0=tpwE[:, C:2 * C],
                                            in1=xs, op=OP.mult)
                    nc.vector.tensor_tensor(out=Mr[:, C:2 * C], in0=tpwE[:, 0:C],
                                            in1=xs, op=OP.mult)
                    nc.vector.tensor_tensor(
                        out=Mr[:, 2 * C:11 * C].rearrange(
                            "p (w r c) -> p w r c", w=3, r=3),
                        in0=tpwE[:, 2 * C:5 * C].rearrange(
                            "p (w c) -> p w c", w=3).unsqueeze(2).to_broadcast(
                            [128, 3, 3, C]),
                        in1=xv.rearrange("p (r c) -> p r c", r=3).unsqueeze(
                            1).to_broadcast([128, 3, 3, C]),
                        op=OP.mult)

                    # segment matmuls with scaled one-hots
                    for (s7i, rc, pc, nch, first) in SEG:
                        nc.tensor.matmul(
                            out=pmsg[:, pc * C:(pc + nch) * C],
                            lhsT=S_sb[:, (s7i * TPB + t) * 128:
                                      (s7i * TPB + t + 1) * 128],
                            rhs=Mr[:, rc * C:(rc + nch) * C],
                            start=(t == 0 and first), stop=(t == TPB - 1),
                            skip_group_check=True)

            # ---- block tail: evict msg, transpose, output linear ----
            msg_sb = tp_blk.tile([128, 11 * C], dt.bfloat16, tag="msg")
            nc.vector.tensor_copy(out=msg_sb[:, 0:6 * C], in_=pmsg[:, 0:6 * C])
            nc.scalar.copy(out=msg_sb[:, 6 * C:11 * C], in_=pmsg[:, 6 * C:11 * C])
            msgT = tp_blk.tile([128, 11 * C], dt.bfloat16, tag="msgT")
            for j in range(11):
                ptr = ps.tile([128, 512], dt.bfloat16, tag="sm")
                nc.tensor.transpose(out=ptr[:, 0:C],
                                    in_=msg_sb[:, j * C:(j + 1) * C],
                                    identity=ident[:])
                if j % 2 == 0:
                    nc.vector.tensor_copy(out=msgT[:, j * C:(j + 1) * C],
                                          in_=ptr[:, 0:C])
                else:
                    nc.scalar.copy(out=msgT[:, j * C:(j + 1) * C], in_=ptr[:, 0:C])
            outmsg = tp_blk.tile([128, 4 * C], dt.float32, tag="outmsg")
            CH = {0: (0, 6), 1: (4, 1, 10), 2: (7, 2, 5), 3: (9, 3, 8)}
            for c4 in range(4):
                chunks = CH[c4]
                pout = ps.tile([128, 512], dt.float32, tag="sm")
                for ji, j in enumerate(chunks):
                    nc.tensor.matmul(
                        out=pout[:, 0:C],
                        lhsT=wout_sb[:, j * C:(j + 1) * C],
                        rhs=msgT[:, j * C:(j + 1) * C],
                        start=(ji == 0), stop=(ji == len(chunks) - 1))
                if c4 % 2 == 0:
                    nc.vector.tensor_copy(out=outmsg[:, c4 * C:(c4 + 1) * C],
                                          in_=pout[:, 0:C])
                else:
                    nc.scalar.copy(out=outmsg[:, c4 * C:(c4 + 1) * C],
                                   in_=pout[:, 0:C])
            nc.sync.dma_start(out=msg_out[b].transpose([1, 0, 2]),
                              in_=outmsg[:].rearrange("p (c n) -> p c n", c=4))

            # ---- skip connection for this block ----
            outsc = tp_blk.tile([128, 4 * C], dt.float32, tag="outsc")
            for c4 in range(4):
                X = tp_blk.tile([C, A * 128], dt.bfloat16, tag="X")
                nfb = nfown_sb[:, (c4 * NBLK + b) * 128:(c4 * NBLK + b + 1) * 128]
                nc.gpsimd.tensor_tensor(
                    out=X[:].rearrange("p (a n) -> p a n", a=A),
                    in0=nfb.unsqueeze(1).to_broadcast([C, A, 128]),
                    in1=nab_all[:, b * A * 128:(b + 1) * A * 128].rearrange(
                        "p (a n) -> p a n", a=A),
                    op=OP.mult)
                psc = ps.tile([128, 512], dt.float32, tag="sm")
                wsc_c = wsc_sb[:, 0:A * C] if c4 == 0 else wsc_sb[:, A * C:]
                for a in range(A):
                    nc.tensor.matmul(
                        out=psc[:, 0:C],
                        lhsT=X[:, a * 128:(a + 1) * 128],
                        rhs=wsc_c[:, a * C:(a + 1) * C],
                        start=(a == 0), stop=(a == A - 1))
                if c4 % 2 == 0:
                    nc.scalar.copy(out=outsc[:, c4 * C:(c4 + 1) * C],
                                   in_=psc[:, 0:C])
                else:
                    nc.vector.tensor_copy(out=outsc[:, c4 * C:(c4 + 1) * C],
                                          in_=psc[:, 0:C])
            nc.sync.dma_start(out=sc_out[b].transpose([1, 0, 2]),
                              in_=outsc[:].rearrange("p (c o) -> p c o", c=4))

    nc.compile()
    nc.finalize()
    return nc


# ------------------------------------------------------------- pjrt runner
def _prepare_fn(nc):
    """Build a reusable jitted shard_map callable over the bass program."""
    jax, _ = _get_jax()
    from jax.sharding import Mesh, PartitionSpec
    try:
        from jax.experimental.shard_map import shard_map
    except ImportError:
        from jax.shard_map import shard_map
    from concourse import bass2jax, mybir
    bass2jax.install_neuronx_cc_hook()

    partition_name = (nc.partition_id_tensor.name
                      if nc.partition_id_tensor else None)
    in_names, out_names, out_avals, zero_shapes = [], [], [], []
    for alloc in nc.m.functions[0].allocations:
        if not isinstance(alloc, mybir.MemoryLocationSet):
            continue
        name = alloc.memorylocations[0].name
        if alloc.kind == "ExternalInput":
            if name != partition_name:
                in_names.append(name)
        elif alloc.kind == "ExternalOutput":
            shape = tuple(alloc.tensor_shape)
            dtype = mybir.dt.np(alloc.dtype)
            out_names.append(name)
            out_avals.append(jax.core.ShapedArray(shape, dtype))
            zero_shapes.append((shape, dtype))
    all_in = list(in_names) + list(out_names)
    if partition_name is not None:
        all_in.append(partition_name)

    def _body(*args):
        operands = list(args)
        if partition_name is not None:
            operands.append(bass2jax.partition_id_tensor())
        outs = bass2jax._bass_exec_p.bind(
            *operands,
            out_avals=tuple(out_avals),
            in_names=tuple(all_in),
            out_names=tuple(out_names),
            lowering_input_output_aliases=(),
            sim_require_finite=False,
            sim_require_nnan=False,
            nc=nc,
        )
        return tuple(outs)

    devices = jax.devices()[:NCORES]
    mesh = Mesh(np.asarray(devices), ("core",))
    nin = len(in_names) + len(zero_shapes)
    fn = jax.jit(shard_map(
        _body, mesh=mesh,
        in_specs=(PartitionSpec("core"),) * nin,
        out_specs=(PartitionSpec("core"),) * len(out_names),
        check_rep=False))
    return fn, in_names, out_names, zero_shapes


def _concat_args(in_maps, in_names, zero_shapes):
    concat_in = [
        np.concatenate([np.asarray(in_maps[c][nm]) for c in range(NCORES)], 0)
        for nm in in_names
    ]
    concat_zero = [np.zeros((NCORES * s[0], *s[1:]), d) for s, d in zero_shapes]
    return tuple(concat_in + concat_zero)


# ------------------------------------------------------------------ driver
def _assemble(out_map):
    blob = np.asarray(out_map["out_blob"]).reshape(NCORES, 2, NBLK, 4, 128, 128)
    msg_out = blob[:, 0]
    sc_out = blob[:, 1]
    message = np.zeros((N, 4 * C), np.float32)
    sc = np.zeros((N, 4 * C), np.float32)
    for k in range(NCORES):
        for b in range(NBLK):
            n0 = k * NPC + b * 128
            n1 = min(n0 + 128, (k + 1) * NPC)
            nn = n1 - n0
            message[n0:n1, 0:C] = msg_out[k, b, 0, :, :nn].T
            for i in range(3):
                message[n0:n1, C + i::3] = msg_out[k, b, 1 + i, :, :nn].T
            sc[n0:n1, 0:C] = sc_out[k, b, 0, :nn, :]
            for i in range(3):
                sc[n0:n1, C + i::3] = sc_out[k, b, 1 + i, :nn, :]
    return message, sc


def _numpy_fallback(inp):
    na = np.asarray(inp["node_attrs"], np.float32)
    nfs = np.asarray(inp["node_feats_s"], np.float32)
    nfv = np.asarray(inp["node_feats_v"], np.float32)
    ea = np.asarray(inp["edge_attrs"], np.float32)
    ef = np.asarray(inp["edge_feats"], np.float32)
    snd = np.asarray(inp["senders"]).astype(np.int64)
    rcv = np.asarray(inp["receivers"]).astype(np.int64)
    inv = np.float32(1.0 / np.sqrt(C * A))
    invc = np.float32(1.0 / np.sqrt(C))
    tp_s = (nfs[:, :, None] * na[:, None, :]).reshape(N, C * A)
    sc_s = tp_s @ np.asarray(inp["W_sc_s"], np.float32) * inv
    tp_v = (nfv[:, :, None, :] * na[:, None, :, None]).reshape(N, C * A, 3)
    sc_v = np.einsum("nki,ko->noi", tp_v,
                     np.asarray(inp["W_sc_v"], np.float32)) * inv
    x_s = nfs @ np.asarray(inp["W_lin_s"], np.float32) * invc
    x_v = np.einsum("nci,co->noi", nfv, np.asarray(inp["W_lin_v"], np.float32)) * invc

    def silu(x):
        return x / (1.0 + np.exp(-x))
    h = silu(ef @ np.asarray(inp["mlp_w0"], np.float32) / np.sqrt(np.float32(F)))
    h = silu(h @ np.asarray(inp["mlp_w1"], np.float32) / np.sqrt(np.float32(H)))
    h = silu(h @ np.asarray(inp["mlp_w2"], np.float32) / np.sqrt(np.float32(H)))
    tpw = h @ np.asarray(inp["mlp_w3"], np.float32) / np.sqrt(np.float32(H))
    w1, w2, w3, w4, w5 = np.split(tpw, 5, axis=1)
    xs, xv = x_s[snd], x_v[snd]
    es, ev = ea[:, 0:1], ea[:, 1:4]
    m0a = w1 * xs * es
    m1a = (w2 * xs)[:, :, None] * ev[:, None, :]
    m1b = w3[:, :, None] * xv * es[:, :, None]
    m0b = w4 * np.einsum("eci,ei->ec", xv, ev) / np.sqrt(np.float32(3))
    m1c = w5[:, :, None] * np.cross(xv, ev[:, None, :]) / np.sqrt(np.float32(2))
    mid_s = np.concatenate([m0a, m0b], axis=1)
    mid_v = np.concatenate([m1a, m1b, m1c], axis=1)
    msg_s = np.zeros((N, 2 * C), np.float32)
    np.add.at(msg_s, rcv, mid_s)
    msg_v = np.zeros((N, 3 * C, 3), np.float32)
    np.add.at(msg_v, rcv, mid_v)
    out_s = (msg_s @ np.asarray(inp["W_out_s"], np.float32)
             / np.sqrt(np.float32(2 * C)) / AVG_NEIGH)
    out_v = (np.einsum("nki,ko->noi", msg_v, np.asarray(inp["W_out_v"], np.float32))
             / np.sqrt(np.float32(3 * C)) / AVG_NEIGH)
    message = np.concatenate([out_s, out_v.reshape(N, C * 3)], axis=1)
    sc = np.concatenate([sc_s, sc_v.reshape(N, C * 3)], axis=1)
    return message.astype(np.float32), sc.astype(np.float32)


def kernel(node_attrs, node_feats_s, node_feats_v, edge_attrs, edge_feats,
           W_sc_s, W_sc_v, W_lin_s, W_lin_v,
           mlp_w0, mlp_w1, mlp_w2, mlp_w3,
           W_out_s, W_out_v, senders, receivers):
    inp = dict(node_attrs=node_attrs, node_feats_s=node_feats_s,
               node_feats_v=node_feats_v, edge_attrs=edge_attrs,
               edge_feats=edge_feats, W_sc_s=W_sc_s, W_sc_v=W_sc_v,
               W_lin_s=W_lin_s, W_lin_v=W_lin_v, mlp_w0=mlp_w0, mlp_w1=mlp_w1,
               mlp_w2=mlp_w2, mlp_w3=mlp_w3, W_out_s=W_out_s, W_out_v=W_out_v,
               senders=senders, receivers=receivers)
    try:
        in_maps = _host_prep(inp)
        if in_maps is None:
            raise RuntimeError("edge tile overflow; falling back")
        if "nc" not in _cache:
            _cache["nc"] = _build_program()
        if "fn" not in _cache:
            fn, in_names, out_names, zero_shapes = _prepare_fn(_cache["nc"])
            _cache.update(fn=fn, in_names=in_names, out_names=out_names,
                          zero_shapes=zero_shapes)
        args = _concat_args(in_maps, _cache["in_names"], _cache["zero_shapes"])
        out = _cache["fn"](*args)
        out = [np.asarray(o) for o in out]
        _capture["fn"] = _cache["fn"]
        _capture["args"] = args
        out_map = {nm: out[i] for i, nm in enumerate(_cache["out_names"])}
        return _assemble(out_map)
    except Exception:
        import traceback
        traceback.print_exc()
        return _numpy_fallback(inp)


if __name__ == "__main__":
    import jax as _j
    with _j.default_device(_j.devices("cpu")[0]):
        import reference
        inputs = {k: np.asarray(v) for k, v in reference.setup_inputs().items()}
        exp_msg, exp_sc = (np.asarray(x) for x in reference.reference(**inputs))
    act_msg, act_sc = kernel(**inputs)
    for name, e, a in (("message", exp_msg, act_msg), ("sc", exp_sc, act_sc)):
        err = np.abs(a - e).max() / (np.abs(e).max() + 1e-9)
        print(f"{name}: rel_err={err:.3e}", flush=True)

